# revision 31
# baseline (speedup 1.0000x reference)
"""Trainium2 Bass kernel: fused multi-head attention (B=4, S=2048, E=1024, H=16).

Sharding: 8 cores = 4 batches x 2 head-groups (data parallel on batch,
tensor parallel on heads). Each core computes attention for one batch and
8 heads, plus its partial output projection; the host sums the two partial
projections per batch (the tensor-parallel reduce) and adds b_out.

Per-core dataflow (all feature-major, zero on-chip transposes):
  v projection    -> v_sb  [s=2048, 8*(64+1)] (s-major; 65th column per head = 1.0)
  k projection    -> k_sb  [pair-tile p: k of heads 2p|2p+1, full s range]
  q projection    -> per s-chunk rotating tiles, interleaved with attention
                     so the exp stream starts while projections still run
  logits^T        =  k_tile.T @ q -> PSUM [t,s]; head A on PE rows 0-63 and
                     head B on rows 64-127 issue adjacently (concurrent
                     row-group matmuls); exp on ACT -> SBUF
  PV              =  [v | 1].T @ exp -> PSUM [65, s]: rows 0-63 raw out^T,
                     row 64 = Z (softmax denominator, no extra pass)
  normalize       :  valsT = raw * (1/Z) broadcast (GPSIMD partition_broadcast)
  out projection  =  valsT.T @ W_outT -> y partial [s, 1024]

Softmax skips max-subtraction: |logits/8| <= ~7 for this problem, exp is
safely in fp32 range and the result is mathematically identical.

Matmul dtypes are per-stage knobs: float32 (exact, 4 cyc/row on the PE) or
float32r (fast path, 1 cyc/row at N>=256, TF32-like operand rounding).
Walrus requires every producer of an f32r matmul operand to emit f32r, so
the stage dtype is threaded through the DRAM decls and SBUF tiles.
"""

import numpy as np
import concourse.bacc as bacc
import concourse.mybir as mybir
import concourse.tile as tile
from concourse.bass_utils import run_bass_kernel_spmd

B, S, D = 4, 2048, 1024
H, HD, E = 16, 64, 1024
HPC = 8            # heads per core
NPAIR = HPC // 2   # head pairs per core
N_CORES = 8
FP = mybir.dt.float32

MM_PROJ = mybir.dt.float32r
MM_ATTN = mybir.dt.float32r
MM_OUT = mybir.dt.float32r

_cache = {}


def _build_program(with_bias: bool,
                   mm_proj=None, mm_attn=None, mm_out=None):
    mm_proj = mm_proj or MM_PROJ
    mm_attn = mm_attn or MM_ATTN
    mm_out = mm_out or MM_OUT
    nc = bacc.Bacc("TRN2", target_bir_lowering=False, debug=False,
                   num_devices=N_CORES)
    xT = nc.dram_tensor("xT", [D, S], mm_proj, kind="ExternalInput").ap()
    wq = nc.dram_tensor("wq", [D, HPC * HD], mm_proj,
                        kind="ExternalInput").ap()
    wk = nc.dram_tensor("wk", [D, HPC * HD], mm_proj,
                        kind="ExternalInput").ap()
    wv = nc.dram_tensor("wv", [D, HPC * HD], mm_proj,
                        kind="ExternalInput").ap()
    wout = nc.dram_tensor("wout", [HPC * HD, E], mm_out,
                          kind="ExternalInput").ap()
    bq = bk = bv = None
    if with_bias:
        bq = nc.dram_tensor("bq", [HPC * HD], FP, kind="ExternalInput").ap()
        bk = nc.dram_tensor("bk", [HPC * HD], FP, kind="ExternalInput").ap()
        bv = nc.dram_tensor("bv", [HPC * HD], FP, kind="ExternalInput").ap()
    y = nc.dram_tensor("y", [S, E], FP, kind="ExternalOutput").ap()

    Exp = mybir.ActivationFunctionType.Exp
    mult = mybir.AluOpType.mult
    SC = S // 512  # 4 s-chunks of 512

    def load_cols(pool, name, src, n):
        cols = []
        for i in range(n):
            bc = pool.tile([128, 1], FP, name=f"{name}{i}")
            nc.sync.dma_start(bc, src[i * 128:(i + 1) * 128].rearrange(
                "(p o) -> p o", o=1))
            cols.append(bc)
        return cols

    with tile.TileContext(nc) as tc:
        with tc.tile_pool(name="persist", bufs=1) as pp, \
             tc.tile_pool(name="xtp", bufs=16) as xtp, \
             tc.tile_pool(name="qrot", bufs=8) as qrot, \
             tc.tile_pool(name="pjps2", bufs=1, space="PSUM") as pjps2:
            k_sb = [pp.tile([128, S], mm_attn, name=f"k{i}")
                    for i in range(NPAIR)]
            v_sb = [pp.tile([128, HPC * 65], mm_attn, name=f"v{i}")
                    for i in range(16)]
            wout_sb = [pp.tile([128, E], mm_out, name=f"wo{i}")
                       for i in range(4)]
            for i in range(16):
                v3 = v_sb[i].rearrange("p (h c) -> p h c", c=65)
                nc.vector.memset(v3[:, :, 64:65].bitcast(FP), 1.0)
            bq_cols = bk_cols = bv_cols = None
            if with_bias:
                bq_cols = load_cols(pp, "bq", bq, 4)
                bk_cols = load_cols(pp, "bk", bk, 4)
                bv_cols = load_cols(pp, "bv", bv, 4)

            wq_sb = [pp.tile([128, 512], mm_proj, name=f"wq{i}")
                     for i in range(8)]

            # ---- Pass 1: v and k projections, streaming xT per s-chunk ----
            with tc.tile_pool(name="wvkp", bufs=1) as wvkp, \
                 tc.tile_pool(name="pjps1", bufs=4, space="PSUM") as pjps1:
                wv_sb = [wvkp.tile([128, 512], mm_proj, name=f"wv{i}")
                         for i in range(8)]
                wk_sb = [wvkp.tile([128, 512], mm_proj, name=f"wk{i}")
                         for i in range(8)]
                # PE warmup spanning the whole initial DMA window (~10us):
                # sustained dummy matmuls release the HAM clock throttle and
                # keep it released until the first real matmul (an idle gap
                # >3.4us would re-throttle the PE to half clock).
                warm = pp.tile([128, 512], mm_proj, name="warm")
                nc.vector.memset(warm.bitcast(FP), 0.0)
                wps = pjps1.tile([128, 512], FP, name="wps", tag="pj")
                for i in range(44):
                    nc.tensor.matmul(wps, lhsT=warm[:, 0:128], rhs=warm,
                                     start=True, stop=True)
                xt0 = []
                for d in range(8):
                    nc.sync.dma_start(wv_sb[d], wv[d * 128:(d + 1) * 128, :])
                    t = xtp.tile([128, 512], mm_proj, name="xt", tag="xt")
                    nc.sync.dma_start(t, xT[d * 128:(d + 1) * 128, 0:512])
                    xt0.append(t)
                for i in range(8):
                    nc.sync.dma_start(wk_sb[i], wk[i * 128:(i + 1) * 128, :])
                for i in range(8):
                    nc.sync.dma_start(wq_sb[i], wq[i * 128:(i + 1) * 128, :])
                for sc in range(SC):
                    s0 = sc * 512
                    if sc == 0:
                        xt = xt0
                    else:
                        xt = []
                        for d in range(8):
                            t = xtp.tile([128, 512], mm_proj, name="xt",
                                         tag="xt")
                            nc.sync.dma_start(t, xT[d * 128:(d + 1) * 128,
                                                    s0:s0 + 512])
                            xt.append(t)
                    for st in range(4):
                        ps = pjps1.tile([128, 512], FP, name="pjv", tag="pj")
                        for d in range(8):
                            nc.tensor.matmul(
                                ps,
                                lhsT=xt[d][:, st * 128:(st + 1) * 128],
                                rhs=wv_sb[d],
                                start=(d == 0), stop=(d == 7))
                        s_tile = sc * 4 + st
                        dst = v_sb[s_tile].rearrange(
                            "p (h c) -> p h c", c=65)[:, :, 0:64]
                        src = ps.rearrange("p (h c) -> p h c", c=64)
                        nc.vector.tensor_copy(dst, src)
                        if with_bias:
                            # v bias is added after normalization (commutes).
                            pass
                    for p in range(NPAIR):
                        ps = pjps1.tile([128, 512], FP, name="pjk", tag="pj")
                        for d in range(8):
                            nc.tensor.matmul(
                                ps,
                                lhsT=wk_sb[d][:, p * 128:(p + 1) * 128],
                                rhs=xt[d],
                                start=(d == 0), stop=(d == 7))
                        dst = k_sb[p][:, s0:s0 + 512]
                        if with_bias:
                            nc.vector.tensor_scalar_add(dst, ps, bk_cols[p])
                        else:
                            nc.vector.tensor_copy(dst, ps)
                    if sc == 0:
                        # wout is not needed until the first output
                        # projection (~100us in); load it off the hot path.
                        for i in range(4):
                            nc.sync.dma_start(wout_sb[i],
                                              wout[i * 128:(i + 1) * 128, :])

            def qproj(sc):
                s0 = sc * 512
                xt = []
                for d in range(8):
                    t = xtp.tile([128, 512], mm_proj, name="xt", tag="xt")
                    nc.sync.dma_start(t, xT[d * 128:(d + 1) * 128,
                                            s0:s0 + 512])
                    xt.append(t)
                qts = []
                for p in range(NPAIR):
                    ps = pjps2.tile([128, 512], FP, name="pjq", tag="pj")
                    for d in range(8):
                        nc.tensor.matmul(
                            ps,
                            lhsT=wq_sb[d][:, p * 128:(p + 1) * 128],
                            rhs=xt[d],
                            start=(d == 0), stop=(d == 7))
                    qt = qrot.tile([128, 512], mm_attn, name="qt",
                                   tag="qt")
                    if with_bias:
                        nc.vector.tensor_scalar_add(qt, ps, bq_cols[p])
                    else:
                        nc.vector.tensor_copy(qt, ps)
                    qts.append(qt)
                return qts

            # First chunk's q-projection emitted ahead of the attention
            # block so the first exp starts as soon as pass 1 drains.
            first_qts = qproj(0)

            # ---- Pass 2: q projection interleaved with attention ----
            with tc.tile_pool(name="valsp", bufs=1) as valsp, \
                 tc.tile_pool(name="expp", bufs=4) as expp, \
                 tc.tile_pool(name="recipp", bufs=2) as recipp, \
                 tc.tile_pool(name="ysb", bufs=2) as ysbp, \
                 tc.tile_pool(name="lps", bufs=2, space="PSUM") as lps, \
                 tc.tile_pool(name="pvps", bufs=3, space="PSUM") as pvps:
                valsT = [valsp.tile([128, S], mm_out, name=f"vals{i}")
                        for i in range(NPAIR)]

                def outproj(sc):
                    for st in range(4):
                        s_tile = sc * 4 + st
                        for ec in range(2):
                            yp = pvps.tile([128, 512], FP, name="yps",
                                           tag="pv")
                            for c in range(4):
                                nc.tensor.matmul(
                                    yp,
                                    lhsT=valsT[c][:, s_tile * 128:
                                                  (s_tile + 1) * 128],
                                    rhs=wout_sb[c][:, ec * 512:(ec + 1) * 512],
                                    start=(c == 0), stop=(c == 3))
                            yt = ysbp.tile([128, 512], FP, name="yt",
                                           tag="yt")
                            nc.vector.tensor_copy(yt, yp)
                            nc.sync.dma_start(
                                y[s_tile * 128:(s_tile + 1) * 128,
                                  ec * 512:(ec + 1) * 512], yt)

                next_qts = first_qts
                pending = None
                for sc in range(SC):
                    s0 = sc * 512
                    qts = next_qts
                    for p in range(NPAIR):
                        qt, kt = qts[p], k_sb[p]
                        pv_ps = [pvps.tile([128, 512], FP, name="pvps",
                                           tag="pv")[0:65, :]
                                 for _ in range(2)]
                        for g in range(8):  # groups of 2 t-tiles
                            # Head A on PE rows 0-63, head B on rows 64-127;
                            # A/B matmuls issue adjacently so the disjoint
                            # row groups run concurrently.
                            lt = [lps.tile([128, 1024], FP, name="lps",
                                           tag="l") for _ in range(2)]
                            for j in range(2):
                                tt = g * 2 + j
                                for hh in range(2):
                                    pb = hh * 64
                                    nc.tensor.matmul(
                                        lt[hh][:, j * 512:(j + 1) * 512],
                                        lhsT=kt[pb:pb + 64,
                                                tt * 128:(tt + 1) * 128],
                                        rhs=qt[pb:pb + 64, :],
                                        start=True, stop=True)
                            ets = []
                            for hh in range(2):
                                et = expp.tile([128, 1024], mm_attn,
                                               name="et", tag="e")
                                nc.scalar.activation(et, lt[hh], Exp,
                                                     scale=0.125)
                                ets.append(et)
                            for hh in range(2):
                                hidx = (2 * p + hh) * 65
                                for j in range(2):
                                    tt = g * 2 + j
                                    nc.tensor.matmul(
                                        pv_ps[hh],
                                        lhsT=v_sb[tt][:, hidx:hidx + 65],
                                        rhs=ets[hh][:, j * 512:(j + 1) * 512],
                                        start=(g == 0 and j == 0),
                                        stop=(g == 7 and j == 1))
                        for hh in range(2):
                            rp = recipp.tile([1, 512], FP, name="rp", tag="r")
                            nc.vector.reciprocal(rp, pv_ps[hh][64:65, :])
                            rb = recipp.tile([64, 512], FP, name="rb",
                                             tag="rb")
                            nc.gpsimd.partition_broadcast(rb, rp)
                            dst = valsT[p][hh * 64:hh * 64 + 64, s0:s0 + 512]
                            nc.vector.tensor_tensor(
                                dst, pv_ps[hh][0:64, :], rb, mult)
                        if with_bias:
                            nc.vector.tensor_scalar_add(
                                valsT[p][:, s0:s0 + 512],
                                valsT[p][:, s0:s0 + 512],
                                bv_cols[p])
                        if p == 0 and pending is not None:
                            # Previous chunk's output projection, emitted
                            # after the next chunk's first attention pair is
                            # in flight so ACT refills with exp work first.
                            outproj(pending)
                            pending = None
                        if p == 1 and sc + 1 < SC:
                            # Next chunk's q-projection two pairs early, so
                            # its eviction is done before this chunk's last
                            # exp drains (kills the boundary ACT stall).
                            next_qts = qproj(sc + 1)
                    pending = sc
                outproj(SC - 1)
    nc.compile()
    return nc


def _get_nc(with_bias: bool):
    key = (with_bias, MM_PROJ, MM_ATTN, MM_OUT)
    if key not in _cache:
        _cache[key] = _build_program(with_bias)
    return _cache[key]


def _make_in_maps(x, W_qkv, b_qkv):
    """Per-core input shards. Core c = batch c//2, head-group c%2.

    W_qkv row layout from the reference's einops grouping '(h c d)':
    row = h*192 + c*64 + d with c in (0=q, 1=k, 2=v).
    Per-core feature order is pair-major: pair p covers heads
    (g*8+2p, g*8+2p+1); within a 128-wide pair tile, head 2p occupies
    partitions 0-63 and head 2p+1 partitions 64-127.
    """
    x = np.ascontiguousarray(np.asarray(x, np.float32))
    W_qkv = np.asarray(W_qkv, np.float32)
    b_qkv = np.asarray(b_qkv, np.float32)
    with_bias = bool(np.any(b_qkv))

    group = []
    for g in range(2):
        idx = {0: [], 1: [], 2: []}
        for p in range(NPAIR):
            for comp in range(3):
                for h in (g * HPC + 2 * p, g * HPC + 2 * p + 1):
                    idx[comp] += list(range(h * 192 + comp * 64,
                                            h * 192 + comp * 64 + 64))
        group.append({k: np.array(v) for k, v in idx.items()})

    in_maps = []
    for c in range(N_CORES):
        b, g = divmod(c, 2)
        idx = group[g]
        m = {
            "xT": np.ascontiguousarray(x[b].T),
            "wq": np.ascontiguousarray(W_qkv[idx[0], :].T),
            "wk": np.ascontiguousarray(W_qkv[idx[1], :].T),
            "wv": np.ascontiguousarray(W_qkv[idx[2], :].T),
        }
        if with_bias:
            m["bq"] = np.ascontiguousarray(b_qkv[idx[0]])
            m["bk"] = np.ascontiguousarray(b_qkv[idx[1]])
            m["bv"] = np.ascontiguousarray(b_qkv[idx[2]])
        in_maps.append(m)
    return in_maps, with_bias


def _add_wout(in_maps, W_out):
    W_out = np.asarray(W_out, np.float32)
    for c, m in enumerate(in_maps):
        g = c % 2
        m["wout"] = np.ascontiguousarray(W_out[:, g * 512:(g + 1) * 512].T)
    return in_maps


def _gather(results, b_out):
    b_out = np.asarray(b_out, np.float32)
    y = np.empty((B, S, E), np.float32)
    for b in range(B):
        y[b] = results[2 * b]["y"] + results[2 * b + 1]["y"]
    y += b_out
    return y


def kernel(x, W_qkv, b_qkv, W_out, b_out):
    in_maps, with_bias = _make_in_maps(x, W_qkv, b_qkv)
    _add_wout(in_maps, W_out)
    nc = _get_nc(with_bias)
    res = run_bass_kernel_spmd(nc, in_maps, list(range(N_CORES)))
    return _gather(res.results, b_out)


# revision 34
# speedup vs baseline: 1.5103x; 1.5103x over previous
"""Trainium2 Bass kernel: fused multi-head attention (B=4, S=2048, E=1024, H=16).

Sharding: 8 cores = 4 batches x 2 head-groups (data parallel on batch,
tensor parallel on heads). Each core computes attention for one batch and
8 heads, plus its partial output projection; the host sums the two partial
projections per batch (the tensor-parallel reduce) and adds b_out.

Per-core dataflow (all feature-major, zero on-chip transposes):
  v projection    -> v_sb  [s=2048, 8*(64+1)] (s-major; 65th column per head = 1.0)
  k projection    -> k_sb  [pair-tile p: k of heads 2p|2p+1, full s range]
  q projection    -> per s-chunk rotating tiles, interleaved with attention
                     so the exp stream starts while projections still run
  logits^T        =  k_tile.T @ q -> PSUM [t,s]; head A on PE rows 0-63 and
                     head B on rows 64-127 issue adjacently (concurrent
                     row-group matmuls); exp on ACT -> SBUF
  PV              =  [v | 1].T @ exp -> PSUM [65, s]: rows 0-63 raw out^T,
                     row 64 = Z (softmax denominator, no extra pass)
  normalize       :  valsT = raw * (1/Z) broadcast (GPSIMD partition_broadcast)
  out projection  =  valsT.T @ W_outT -> y partial [s, 1024]

Softmax skips max-subtraction: |logits/8| <= ~7 for this problem, exp is
safely in fp32 range and the result is mathematically identical.

Matmul dtypes are per-stage knobs: float32 (exact, 4 cyc/row on the PE) or
float32r (fast path, 1 cyc/row at N>=256, TF32-like operand rounding).
Walrus requires every producer of an f32r matmul operand to emit f32r, so
the stage dtype is threaded through the DRAM decls and SBUF tiles.
"""

import numpy as np
import concourse.bacc as bacc
import concourse.mybir as mybir
import concourse.tile as tile
from concourse.bass_utils import run_bass_kernel_spmd

B, S, D = 4, 2048, 1024
H, HD, E = 16, 64, 1024
HPC = 8            # heads per core
NPAIR = HPC // 2   # head pairs per core
N_CORES = 8
FP = mybir.dt.float32

MM_PROJ = mybir.dt.float32r
MM_ATTN = mybir.dt.float32r
MM_OUT = mybir.dt.float32r

_cache = {}


def _build_program(with_bias: bool,
                   mm_proj=None, mm_attn=None, mm_out=None):
    mm_proj = mm_proj or MM_PROJ
    mm_attn = mm_attn or MM_ATTN
    mm_out = mm_out or MM_OUT
    nc = bacc.Bacc("TRN2", target_bir_lowering=False, debug=False,
                   num_devices=N_CORES)
    xT = nc.dram_tensor("xT", [D, S], mm_proj, kind="ExternalInput").ap()
    wq = nc.dram_tensor("wq", [D, HPC * HD], mm_proj,
                        kind="ExternalInput").ap()
    wk = nc.dram_tensor("wk", [D, HPC * HD], mm_proj,
                        kind="ExternalInput").ap()
    wv = nc.dram_tensor("wv", [D, HPC * HD], mm_proj,
                        kind="ExternalInput").ap()
    wout = nc.dram_tensor("wout", [HPC * HD, E], mm_out,
                          kind="ExternalInput").ap()
    bq = bk = bv = None
    if with_bias:
        bq = nc.dram_tensor("bq", [HPC * HD], FP, kind="ExternalInput").ap()
        bk = nc.dram_tensor("bk", [HPC * HD], FP, kind="ExternalInput").ap()
        bv = nc.dram_tensor("bv", [HPC * HD], FP, kind="ExternalInput").ap()
    y = nc.dram_tensor("y", [S, E], FP, kind="ExternalOutput").ap()

    Exp = mybir.ActivationFunctionType.Exp
    mult = mybir.AluOpType.mult
    SC = S // 512  # 4 s-chunks of 512

    def load_cols(pool, name, src, n):
        cols = []
        for i in range(n):
            bc = pool.tile([128, 1], FP, name=f"{name}{i}")
            nc.sync.dma_start(bc, src[i * 128:(i + 1) * 128].rearrange(
                "(p o) -> p o", o=1))
            cols.append(bc)
        return cols

    with tile.TileContext(nc) as tc:
        with tc.tile_pool(name="persist", bufs=1) as pp, \
             tc.tile_pool(name="xtp", bufs=16) as xtp, \
             tc.tile_pool(name="qrot", bufs=8) as qrot, \
             tc.tile_pool(name="pjps2", bufs=1, space="PSUM") as pjps2:
            k_sb = [pp.tile([128, S], mm_attn, name=f"k{i}")
                    for i in range(NPAIR)]
            v_sb = [pp.tile([128, HPC * 65], mm_attn, name=f"v{i}")
                    for i in range(16)]
            wout_sb = [pp.tile([128, E], mm_out, name=f"wo{i}")
                       for i in range(4)]
            for i in range(16):
                v3 = v_sb[i].rearrange("p (h c) -> p h c", c=65)
                nc.vector.memset(v3[:, :, 64:65].bitcast(FP), 1.0)
            bq_cols = bk_cols = bv_cols = None
            if with_bias:
                bq_cols = load_cols(pp, "bq", bq, 4)
                bk_cols = load_cols(pp, "bk", bk, 4)
                bv_cols = load_cols(pp, "bv", bv, 4)

            wq_sb = [pp.tile([128, 512], mm_proj, name=f"wq{i}")
                     for i in range(8)]

            # ---- Pass 1: v and k projections, streaming xT per s-chunk ----
            with tc.tile_pool(name="wvkp", bufs=1) as wvkp, \
                 tc.tile_pool(name="pjps1", bufs=4, space="PSUM") as pjps1:
                wv_sb = [wvkp.tile([128, 512], mm_proj, name=f"wv{i}")
                         for i in range(8)]
                wk_sb = [wvkp.tile([128, 512], mm_proj, name=f"wk{i}")
                         for i in range(8)]
                # PE warmup spanning the whole initial DMA window (~10us):
                # sustained dummy matmuls release the HAM clock throttle and
                # keep it released until the first real matmul (an idle gap
                # >3.4us would re-throttle the PE to half clock).
                warm = pp.tile([128, 512], mm_proj, name="warm")
                nc.vector.memset(warm.bitcast(FP), 0.0)
                wps = pjps1.tile([128, 512], FP, name="wps", tag="pj")
                for i in range(44):
                    nc.tensor.matmul(wps, lhsT=warm[:, 0:128], rhs=warm,
                                     start=True, stop=True)
                xt0 = []
                for d in range(8):
                    nc.sync.dma_start(wv_sb[d], wv[d * 128:(d + 1) * 128, :])
                    t = xtp.tile([128, 512], mm_proj, name="xt", tag="xt")
                    nc.sync.dma_start(t, xT[d * 128:(d + 1) * 128, 0:512])
                    xt0.append(t)
                for i in range(8):
                    nc.sync.dma_start(wk_sb[i], wk[i * 128:(i + 1) * 128, :])
                for i in range(8):
                    nc.sync.dma_start(wq_sb[i], wq[i * 128:(i + 1) * 128, :])
                for sc in range(SC):
                    s0 = sc * 512
                    if sc == 0:
                        xt = xt0
                    else:
                        xt = []
                        for d in range(8):
                            t = xtp.tile([128, 512], mm_proj, name="xt",
                                         tag="xt")
                            nc.sync.dma_start(t, xT[d * 128:(d + 1) * 128,
                                                    s0:s0 + 512])
                            xt.append(t)
                    for st in range(4):
                        ps = pjps1.tile([128, 512], FP, name="pjv", tag="pj")
                        for d in range(8):
                            nc.tensor.matmul(
                                ps,
                                lhsT=xt[d][:, st * 128:(st + 1) * 128],
                                rhs=wv_sb[d],
                                start=(d == 0), stop=(d == 7))
                        s_tile = sc * 4 + st
                        dst = v_sb[s_tile].rearrange(
                            "p (h c) -> p h c", c=65)[:, :, 0:64]
                        src = ps.rearrange("p (h c) -> p h c", c=64)
                        nc.vector.tensor_copy(dst, src)
                        if with_bias:
                            # v bias is added after normalization (commutes).
                            pass
                    for p in range(NPAIR):
                        ps = pjps1.tile([128, 512], FP, name="pjk", tag="pj")
                        for d in range(8):
                            nc.tensor.matmul(
                                ps,
                                lhsT=wk_sb[d][:, p * 128:(p + 1) * 128],
                                rhs=xt[d],
                                start=(d == 0), stop=(d == 7))
                        dst = k_sb[p][:, s0:s0 + 512]
                        if with_bias:
                            nc.vector.tensor_scalar_add(dst, ps, bk_cols[p])
                        else:
                            nc.vector.tensor_copy(dst, ps)
                    if sc == 0:
                        # wout is not needed until the first output
                        # projection (~100us in); load it off the hot path.
                        for i in range(4):
                            nc.sync.dma_start(wout_sb[i],
                                              wout[i * 128:(i + 1) * 128, :])

            def load_xt(sc):
                xt = []
                for d in range(8):
                    t = xtp.tile([128, 512], mm_proj, name="xt", tag="xt")
                    nc.sync.dma_start(t, xT[d * 128:(d + 1) * 128,
                                            sc * 512:sc * 512 + 512])
                    xt.append(t)
                return xt

            def qproj_chain(p, xt):
                ps = pjps2.tile([128, 512], FP, name="pjq", tag="pj")
                for d in range(8):
                    nc.tensor.matmul(
                        ps,
                        lhsT=wq_sb[d][:, p * 128:(p + 1) * 128],
                        rhs=xt[d],
                        start=(d == 0), stop=(d == 7))
                qt = qrot.tile([128, 512], mm_attn, name="qt", tag="qt")
                if with_bias:
                    nc.vector.tensor_scalar_add(qt, ps, bq_cols[p])
                else:
                    nc.vector.tensor_copy(qt, ps)
                return qt

            # First chunk's q-projection emitted ahead of the attention
            # block so the first exp starts as soon as pass 1 drains.
            _xt0 = load_xt(0)
            first_qts = [qproj_chain(p, _xt0) for p in range(NPAIR)]

            # ---- Pass 2: q projection interleaved with attention ----
            with tc.tile_pool(name="valsp", bufs=1) as valsp, \
                 tc.tile_pool(name="expp", bufs=4) as expp, \
                 tc.tile_pool(name="recipp", bufs=2) as recipp, \
                 tc.tile_pool(name="ysb", bufs=2) as ysbp, \
                 tc.tile_pool(name="lps", bufs=2, space="PSUM") as lps, \
                 tc.tile_pool(name="pvps", bufs=3, space="PSUM") as pvps:
                valsT = [valsp.tile([128, S], mm_out, name=f"vals{i}")
                        for i in range(NPAIR)]

                def outproj_part(sc, st):
                    # One s-tile (two e-chunk groups) of chunk sc's output
                    # projection — emitted one s-tile per attention pair so
                    # the PE insertions stay under ACT's per-pair slack.
                    s_tile = sc * 4 + st
                    for ec in range(2):
                        yp = pvps.tile([128, 512], FP, name="yps",
                                       tag="pv")
                        for c in range(4):
                            nc.tensor.matmul(
                                yp,
                                lhsT=valsT[c][:, s_tile * 128:
                                              (s_tile + 1) * 128],
                                rhs=wout_sb[c][:, ec * 512:(ec + 1) * 512],
                                start=(c == 0), stop=(c == 3))
                        yt = ysbp.tile([128, 512], FP, name="yt",
                                       tag="yt")
                        nc.vector.tensor_copy(yt, yp)
                        nc.sync.dma_start(
                            y[s_tile * 128:(s_tile + 1) * 128,
                              ec * 512:(ec + 1) * 512], yt)

                next_qts = first_qts
                pending = None
                for sc in range(SC):
                    s0 = sc * 512
                    qts = next_qts
                    for p in range(NPAIR):
                        qt, kt = qts[p], k_sb[p]
                        pv_ps = [pvps.tile([128, 512], FP, name="pvps",
                                           tag="pv")[0:65, :]
                                 for _ in range(2)]
                        for g in range(8):  # groups of 2 t-tiles
                            # Head A on PE rows 0-63, head B on rows 64-127;
                            # A/B matmuls issue adjacently so the disjoint
                            # row groups run concurrently.
                            lt = [lps.tile([128, 1024], FP, name="lps",
                                           tag="l") for _ in range(2)]
                            for j in range(2):
                                tt = g * 2 + j
                                for hh in range(2):
                                    pb = hh * 64
                                    nc.tensor.matmul(
                                        lt[hh][:, j * 512:(j + 1) * 512],
                                        lhsT=kt[pb:pb + 64,
                                                tt * 128:(tt + 1) * 128],
                                        rhs=qt[pb:pb + 64, :],
                                        start=True, stop=True)
                            ets = []
                            for hh in range(2):
                                et = expp.tile([128, 1024], mm_attn,
                                               name="et", tag="e")
                                nc.scalar.activation(et, lt[hh], Exp,
                                                     scale=0.125)
                                ets.append(et)
                            for hh in range(2):
                                hidx = (2 * p + hh) * 65
                                for j in range(2):
                                    tt = g * 2 + j
                                    nc.tensor.matmul(
                                        pv_ps[hh],
                                        lhsT=v_sb[tt][:, hidx:hidx + 65],
                                        rhs=ets[hh][:, j * 512:(j + 1) * 512],
                                        start=(g == 0 and j == 0),
                                        stop=(g == 7 and j == 1))
                        for hh in range(2):
                            rp = recipp.tile([1, 512], FP, name="rp", tag="r")
                            nc.vector.reciprocal(rp, pv_ps[hh][64:65, :])
                            rb = recipp.tile([64, 512], FP, name="rb",
                                             tag="rb")
                            nc.gpsimd.partition_broadcast(rb, rp)
                            dst = valsT[p][hh * 64:hh * 64 + 64, s0:s0 + 512]
                            nc.vector.tensor_tensor(
                                dst, pv_ps[hh][0:64, :], rb, mult)
                        if with_bias:
                            nc.vector.tensor_scalar_add(
                                valsT[p][:, s0:s0 + 512],
                                valsT[p][:, s0:s0 + 512],
                                bv_cols[p])
                        # Spread the previous chunk's output projection and
                        # the next chunk's q-projection across the attention
                        # pairs: ~3.5us of PE work per pair fits inside
                        # ACT's per-pair slack, so the exp stream never
                        # starves at chunk boundaries.
                        if pending is not None:
                            outproj_part(pending, p)
                        if sc + 1 < SC:
                            if p == 0:
                                xt_next = load_xt(sc + 1)
                                next_qts = []
                            next_qts.append(qproj_chain(p, xt_next))
                    pending = sc
                for st in range(4):
                    outproj_part(SC - 1, st)
    nc.compile()
    return nc


def _get_nc(with_bias: bool):
    key = (with_bias, MM_PROJ, MM_ATTN, MM_OUT)
    if key not in _cache:
        _cache[key] = _build_program(with_bias)
    return _cache[key]


def _make_in_maps(x, W_qkv, b_qkv):
    """Per-core input shards. Core c = batch c//2, head-group c%2.

    W_qkv row layout from the reference's einops grouping '(h c d)':
    row = h*192 + c*64 + d with c in (0=q, 1=k, 2=v).
    Per-core feature order is pair-major: pair p covers heads
    (g*8+2p, g*8+2p+1); within a 128-wide pair tile, head 2p occupies
    partitions 0-63 and head 2p+1 partitions 64-127.
    """
    x = np.ascontiguousarray(np.asarray(x, np.float32))
    W_qkv = np.asarray(W_qkv, np.float32)
    b_qkv = np.asarray(b_qkv, np.float32)
    with_bias = bool(np.any(b_qkv))

    group = []
    for g in range(2):
        idx = {0: [], 1: [], 2: []}
        for p in range(NPAIR):
            for comp in range(3):
                for h in (g * HPC + 2 * p, g * HPC + 2 * p + 1):
                    idx[comp] += list(range(h * 192 + comp * 64,
                                            h * 192 + comp * 64 + 64))
        group.append({k: np.array(v) for k, v in idx.items()})

    in_maps = []
    for c in range(N_CORES):
        b, g = divmod(c, 2)
        idx = group[g]
        m = {
            "xT": np.ascontiguousarray(x[b].T),
            "wq": np.ascontiguousarray(W_qkv[idx[0], :].T),
            "wk": np.ascontiguousarray(W_qkv[idx[1], :].T),
            "wv": np.ascontiguousarray(W_qkv[idx[2], :].T),
        }
        if with_bias:
            m["bq"] = np.ascontiguousarray(b_qkv[idx[0]])
            m["bk"] = np.ascontiguousarray(b_qkv[idx[1]])
            m["bv"] = np.ascontiguousarray(b_qkv[idx[2]])
        in_maps.append(m)
    return in_maps, with_bias


def _add_wout(in_maps, W_out):
    W_out = np.asarray(W_out, np.float32)
    for c, m in enumerate(in_maps):
        g = c % 2
        m["wout"] = np.ascontiguousarray(W_out[:, g * 512:(g + 1) * 512].T)
    return in_maps


def _gather(results, b_out):
    b_out = np.asarray(b_out, np.float32)
    y = np.empty((B, S, E), np.float32)
    for b in range(B):
        y[b] = results[2 * b]["y"] + results[2 * b + 1]["y"]
    y += b_out
    return y


def kernel(x, W_qkv, b_qkv, W_out, b_out):
    in_maps, with_bias = _make_in_maps(x, W_qkv, b_qkv)
    _add_wout(in_maps, W_out)
    nc = _get_nc(with_bias)
    res = run_bass_kernel_spmd(nc, in_maps, list(range(N_CORES)))
    return _gather(res.results, b_out)
